# revision 5
# baseline (speedup 1.0000x reference)
"""Adaptive Huber/MSE/L1 loss on 8 TRN2 NeuronCores (Bass/Tile).

Reference math (per sample, N = 4,096,000 elements):
    e   = pred - true
    L2  = mean(e^2);  L1 = mean(|e|)
    huber_elem = where(|e| <= 5, 0.5 e^2, 5(|e| - 2.5))
               = 0.5 e^2 - 0.5 relu(|e| - 5)^2
    huber = (S2 - SR) * 0.5 / N        (S2 = sum e^2, SR = sum relu(|e|-5)^2)
    use_l2 = (L2 <= 1) | (L2 < L1^2)
    loss = mean_over_batch(where(use_l2, L2, huber))

Sharding: data-parallel, sample i -> core i. Each core reduces its
32.8 MB shard to three sums, applies the branch locally, scales by
1/8, then a 4-byte AllReduce(add) yields the batch mean on every core.

Engine split per [128, F] tile (memory-bound, ~358 GB/s/core DMA floor):
    DVE : e = a - b;  bn_stats chunks (-> per-partition mean/var => S2)
    ACT : ae = |e| with row-accum (S1);  Square(m - 5) with row-accum (SR)
    GPS : m = max(ae, 5)
    PE  : ones^T @ [128,3] partition reduction into PSUM
(The fused DVE tensor_tensor_reduce path is avoided deliberately: it
compiles and simulates but hangs the device on this toolchain.)
"""

import numpy as np

import concourse.bass as bass
import concourse.bacc as bacc
import concourse.mybir as mybir
from concourse.tile import TileContext
from concourse.bass_utils import run_bass_kernel_spmd

P = 128
COLS = 32000  # 160*160*160 / 128
DELTA = 5.0
N_CORES = 8

F32 = mybir.dt.float32
ALU = mybir.AluOpType
ACTF = mybir.ActivationFunctionType


def build(cols=COLS, tile_f=2000):
    assert cols % tile_f == 0
    n_tiles = cols // tile_f
    chunk = 500 if tile_f % 500 == 0 else tile_f
    assert tile_f % chunk == 0 and chunk <= 512
    n_chunks = tile_f // chunk
    n_elem = float(P * cols)

    nc = bacc.Bacc(
        "TRN2",
        target_bir_lowering=False,
        debug=False,
        enable_asserts=False,
        num_devices=N_CORES,
    )
    a_ext = nc.dram_tensor("y_pred_logits", [P, cols], F32, kind="ExternalInput")
    b_ext = nc.dram_tensor("y_true", [P, cols], F32, kind="ExternalInput")
    out_ext = nc.dram_tensor("out", [1, 1], F32, kind="ExternalOutput")

    with TileContext(nc) as tc:
        with (
            tc.tile_pool(name="io", bufs=3) as io_pool,
            tc.tile_pool(name="work", bufs=2) as work_pool,
            tc.tile_pool(name="acc", bufs=1) as acc_pool,
            tc.tile_pool(name="psum", bufs=1, space="PSUM") as psum_pool,
            tc.tile_pool(name="dram", bufs=1, space="DRAM") as dram_pool,
        ):
            stats = acc_pool.tile([P, n_tiles * n_chunks, 6], F32)
            sums_ae = acc_pool.tile([P, n_tiles], F32)
            sums_d2 = acc_pool.tile([P, n_tiles], F32)
            ones = acc_pool.tile([P, 1], F32)
            nc.vector.memset(ones[:], 1.0)
            neg_delta = acc_pool.tile([P, 1], F32)
            nc.vector.memset(neg_delta[:], -DELTA)

            for t in range(n_tiles):
                a = io_pool.tile([P, tile_f], F32, tag="a")
                b = io_pool.tile([P, tile_f], F32, tag="b")
                sl = slice(t * tile_f, (t + 1) * tile_f)
                nc.sync.dma_start(out=a[:], in_=a_ext[:, sl])
                nc.sync.dma_start(out=b[:], in_=b_ext[:, sl])

                e = work_pool.tile([P, tile_f], F32, tag="e")
                ae = work_pool.tile([P, tile_f], F32, tag="ae")
                m = work_pool.tile([P, tile_f], F32, tag="m")
                s_d2 = work_pool.tile([P, tile_f], F32, tag="s_d2")

                nc.vector.tensor_tensor(e[:], a[:], b[:], ALU.subtract)
                for c in range(n_chunks):
                    nc.vector.bn_stats(
                        out=stats[:, t * n_chunks + c, :],
                        in_=e[:, c * chunk : (c + 1) * chunk],
                    )
                nc.scalar.activation(
                    ae[:], e[:], ACTF.Abs, accum_out=sums_ae[:, t : t + 1]
                )
                nc.gpsimd.tensor_scalar(m[:], ae[:], DELTA, None, ALU.max)
                # Square(m - 5) == relu(|e|-5)^2 since m >= 5
                nc.scalar.activation(
                    s_d2[:], m[:], ACTF.Square, bias=neg_delta[:, 0:1],
                    accum_out=sums_d2[:, t : t + 1],
                )

            # per-partition S2/cols = var + mean^2 from aggregated stats
            mv = acc_pool.tile([P, 2], F32)
            nc.vector.bn_aggr(out=mv[:], in_=stats[:])
            red = acc_pool.tile([P, 4], F32)
            nc.vector.tensor_tensor(red[:, 3:4], mv[:, 0:1], mv[:, 0:1], ALU.mult)
            nc.vector.tensor_tensor(red[:, 0:1], red[:, 3:4], mv[:, 1:2], ALU.add)
            nc.vector.reduce_sum(red[:, 1:2], sums_ae[:], axis=mybir.AxisListType.X)
            nc.vector.reduce_sum(red[:, 2:3], sums_d2[:], axis=mybir.AxisListType.X)

            # partition reduce: ps[0,:] = ones^T @ red -> [S2/cols, S1, SR]
            ps = psum_pool.tile([1, 4], F32)
            nc.tensor.matmul(ps[0:1, 0:3], ones[:, 0:1], red[:, 0:3],
                             start=True, stop=True)
            psc = acc_pool.tile([1, 3], F32)
            nc.scalar.activation(psc[:], ps[0:1, 0:3], ACTF.Copy)

            sc = acc_pool.tile([1, 12], F32)
            l2 = sc[:, 0:1]
            l1 = sc[:, 1:2]
            srn = sc[:, 2:3]
            t0 = sc[:, 3:4]
            hub = sc[:, 4:5]
            l1sq = sc[:, 5:6]
            c1 = sc[:, 6:7]
            c2 = sc[:, 7:8]
            cond = sc[:, 8:9]
            dif = sc[:, 9:10]
            mm = sc[:, 10:11]
            per = sc[:, 11:12]
            res = acc_pool.tile([1, 1], F32)

            # psc0 = sum_p (var_p + mean_p^2) = S2/cols  =>  L2 = psc0/128
            nc.scalar.activation(l2, psc[:, 0:1], ACTF.Copy, scale=1.0 / P)
            nc.scalar.activation(l1, psc[:, 1:2], ACTF.Copy, scale=1.0 / n_elem)
            nc.scalar.activation(srn, psc[:, 2:3], ACTF.Copy, scale=1.0 / n_elem)
            # hub = 0.5 * (L2 - SR/N)
            nc.vector.tensor_tensor(t0, l2, srn, ALU.subtract)
            nc.scalar.activation(hub, t0, ACTF.Copy, scale=0.5)
            nc.vector.tensor_tensor(l1sq, l1, l1, ALU.mult)
            nc.vector.tensor_scalar(c1, l2, 1.0, None, ALU.is_le)
            nc.vector.tensor_tensor(c2, l2, l1sq, ALU.is_lt)
            nc.vector.tensor_tensor(cond, c1, c2, ALU.max)
            # per = hub + cond * (l2 - hub)
            nc.vector.tensor_tensor(dif, l2, hub, ALU.subtract)
            nc.vector.tensor_tensor(mm, cond, dif, ALU.mult)
            nc.vector.tensor_tensor(per, hub, mm, ALU.add)
            nc.scalar.activation(res[:], per, ACTF.Copy, scale=1.0 / N_CORES)

            cc_in = dram_pool.tile([1, 1], F32)
            cc_out = dram_pool.tile([1, 1], F32)
            nc.gpsimd.dma_start(out=cc_in[:], in_=res[:])
            nc.gpsimd.collective_compute(
                "AllReduce",
                ALU.add,
                replica_groups=[list(range(N_CORES))],
                ins=[cc_in.opt()],
                outs=[cc_out.opt()],
            )
            nc.gpsimd.dma_start(out=out_ext[:, :], in_=cc_out[:])

    nc.compile()
    return nc


_NC_CACHE = {}


def _get_nc():
    if "nc" not in _NC_CACHE:
        _NC_CACHE["nc"] = build()
    return _NC_CACHE["nc"]


def kernel(y_pred_logits: np.ndarray, y_true: np.ndarray, _trace=False) -> np.ndarray:
    nc = _get_nc()
    a = np.ascontiguousarray(y_pred_logits, dtype=np.float32).reshape(N_CORES, P, COLS)
    b = np.ascontiguousarray(y_true, dtype=np.float32).reshape(N_CORES, P, COLS)
    in_maps = [
        {"y_pred_logits": a[i], "y_true": b[i]} for i in range(N_CORES)
    ]
    r = run_bass_kernel_spmd(nc, in_maps, core_ids=list(range(N_CORES)), trace=_trace)
    out = np.asarray(r.results[0]["out"], dtype=np.float32).reshape(())
    if _trace:
        return out, r
    return out


# revision 7
# speedup vs baseline: 3.6776x; 3.6776x over previous
"""Adaptive Huber/MSE/L1 loss on 8 TRN2 NeuronCores (Bass/Tile).

Reference math (per sample, N = 4,096,000 elements):
    e   = pred - true
    L2  = mean(e^2);  L1 = mean(|e|)
    huber_elem = where(|e| <= 5, 0.5 e^2, 5(|e| - 2.5))
               = 0.5 e^2 - 0.5 relu(|e| - 5)^2
    huber = (S2 - SR) * 0.5 / N        (S2 = sum e^2, SR = sum relu(|e|-5)^2)
    use_l2 = (L2 <= 1) | (L2 < L1^2)
    loss = mean_over_batch(where(use_l2, L2, huber))

Sharding: data-parallel, sample i -> core i. Each core reduces its
32.8 MB shard to three sums, applies the branch locally, scales by
1/8, then a 4-byte AllReduce(add) yields the batch mean on every core.

Engine split per [128, F] tile, tuned so both compute engines sit just
under the ~358 GB/s-per-core DMA floor (~94 us for 32.8 MB):
    DVE : e = a - b;  m = max(|e|,5)-5 (2x-mode tensor_scalar);
          bn_stats chunks on EVEN tiles (-> mean/var => partial S2)
    ACT : |e| with row-accum (S1);  Square(m)+row-accum (SR);
          Square(e)+row-accum on ODD tiles (other half of S2)
    PE  : ones^T @ [128,4] partition reduction into PSUM

Hardware pitfalls baked in: DVE tensor_tensor_reduce hangs the device
(avoided); GpSimd elementwise runs ~30 us/tile AND port-starves DVE
(avoided); profiling must capture all 8 devices (see test harness).
"""

import numpy as np

import concourse.bass as bass
import concourse.bacc as bacc
import concourse.mybir as mybir
from concourse.tile import TileContext
from concourse.bass_utils import run_bass_kernel_spmd

P = 128
COLS = 32000  # 160*160*160 / 128
DELTA = 5.0
N_CORES = 8

F32 = mybir.dt.float32
ALU = mybir.AluOpType
ACTF = mybir.ActivationFunctionType


def build(cols=COLS, tile_f=2000):
    assert cols % tile_f == 0
    n_tiles = cols // tile_f
    assert n_tiles % 2 == 0
    chunk = 500 if tile_f % 500 == 0 else tile_f
    assert tile_f % chunk == 0 and chunk <= 512
    n_chunks = tile_f // chunk
    n_elem = float(P * cols)
    bn_elems = float((n_tiles // 2) * tile_f)  # per-partition elems seen by bn_stats

    nc = bacc.Bacc(
        "TRN2",
        target_bir_lowering=False,
        debug=False,
        enable_asserts=False,
        num_devices=N_CORES,
    )
    a_ext = nc.dram_tensor("y_pred_logits", [P, cols], F32, kind="ExternalInput")
    b_ext = nc.dram_tensor("y_true", [P, cols], F32, kind="ExternalInput")
    out_ext = nc.dram_tensor("out", [1, 1], F32, kind="ExternalOutput")

    with TileContext(nc) as tc:
        with (
            tc.tile_pool(name="io", bufs=4) as io_pool,
            tc.tile_pool(name="work", bufs=2) as work_pool,
            tc.tile_pool(name="acc", bufs=1) as acc_pool,
            tc.tile_pool(name="psum", bufs=1, space="PSUM") as psum_pool,
            tc.tile_pool(name="dram", bufs=1, space="DRAM") as dram_pool,
        ):
            stats = acc_pool.tile([P, (n_tiles // 2) * n_chunks, 6], F32)
            sums_sq = acc_pool.tile([P, n_tiles // 2], F32)  # odd tiles via ACT
            sums_ae = acc_pool.tile([P, n_tiles], F32)
            sums_d2 = acc_pool.tile([P, n_tiles], F32)
            ones = acc_pool.tile([P, 1], F32)
            nc.vector.memset(ones[:], 1.0)
            neg_delta = acc_pool.tile([P, 1], F32)
            nc.vector.memset(neg_delta[:], -DELTA)

            for t in range(n_tiles):
                a = io_pool.tile([P, tile_f], F32, tag="a")
                b = io_pool.tile([P, tile_f], F32, tag="b")
                sl = slice(t * tile_f, (t + 1) * tile_f)
                nc.sync.dma_start(out=a[:], in_=a_ext[:, sl])
                nc.sync.dma_start(out=b[:], in_=b_ext[:, sl])

                e = work_pool.tile([P, tile_f], F32, tag="e")
                ae = work_pool.tile([P, tile_f], F32, tag="ae")
                m = work_pool.tile([P, tile_f], F32, tag="m")
                s_d2 = work_pool.tile([P, tile_f], F32, tag="s_d2")

                nc.vector.tensor_tensor(e[:], a[:], b[:], ALU.subtract)
                half = t // 2
                if t % 2 == 0:
                    for c in range(n_chunks):
                        nc.vector.bn_stats(
                            out=stats[:, half * n_chunks + c, :],
                            in_=e[:, c * chunk : (c + 1) * chunk],
                        )
                else:
                    s_sq = work_pool.tile([P, tile_f], F32, tag="s_sq")
                    nc.scalar.activation(
                        s_sq[:], e[:], ACTF.Square,
                        accum_out=sums_sq[:, half : half + 1],
                    )
                nc.scalar.activation(
                    ae[:], e[:], ACTF.Abs, accum_out=sums_ae[:, t : t + 1]
                )
                # m = max(|e|,5) - 5 == relu(|e|-5); 2x-mode tensor_scalar
                nc.vector.tensor_scalar(
                    m[:], ae[:], DELTA, -DELTA, ALU.max, ALU.add
                )
                nc.scalar.activation(
                    s_d2[:], m[:], ACTF.Square,
                    accum_out=sums_d2[:, t : t + 1],
                )

            # per-partition S2 = bn_elems*(var + mean^2)  +  sum(odd-tile squares)
            mv = acc_pool.tile([P, 2], F32)
            nc.vector.bn_aggr(out=mv[:], in_=stats[:])
            red = acc_pool.tile([P, 4], F32)
            tmp = acc_pool.tile([P, 2], F32)
            nc.vector.tensor_tensor(tmp[:, 0:1], mv[:, 0:1], mv[:, 0:1], ALU.mult)
            nc.vector.tensor_tensor(tmp[:, 1:2], tmp[:, 0:1], mv[:, 1:2], ALU.add)
            nc.vector.reduce_sum(red[:, 3:4], sums_sq[:], axis=mybir.AxisListType.X)
            # red0 = bn_elems * tmp1 + red3
            nc.vector.tensor_scalar(tmp[:, 1:2], tmp[:, 1:2], bn_elems, None, ALU.mult)
            nc.vector.tensor_tensor(red[:, 0:1], tmp[:, 1:2], red[:, 3:4], ALU.add)
            nc.vector.reduce_sum(red[:, 1:2], sums_ae[:], axis=mybir.AxisListType.X)
            nc.vector.reduce_sum(red[:, 2:3], sums_d2[:], axis=mybir.AxisListType.X)

            # partition reduce: ps[0,:] = ones^T @ red -> [S2, S1, SR]
            ps = psum_pool.tile([1, 4], F32)
            nc.tensor.matmul(ps[0:1, 0:3], ones[:, 0:1], red[:, 0:3],
                             start=True, stop=True)
            psc = acc_pool.tile([1, 3], F32)
            nc.scalar.activation(psc[:], ps[0:1, 0:3], ACTF.Copy)

            sc = acc_pool.tile([1, 12], F32)
            l2 = sc[:, 0:1]
            l1 = sc[:, 1:2]
            srn = sc[:, 2:3]
            t0 = sc[:, 3:4]
            hub = sc[:, 4:5]
            l1sq = sc[:, 5:6]
            c1 = sc[:, 6:7]
            c2 = sc[:, 7:8]
            cond = sc[:, 8:9]
            dif = sc[:, 9:10]
            mm = sc[:, 10:11]
            per = sc[:, 11:12]
            res = acc_pool.tile([1, 1], F32)

            nc.scalar.activation(l2, psc[:, 0:1], ACTF.Copy, scale=1.0 / n_elem)
            nc.scalar.activation(l1, psc[:, 1:2], ACTF.Copy, scale=1.0 / n_elem)
            nc.scalar.activation(srn, psc[:, 2:3], ACTF.Copy, scale=1.0 / n_elem)
            # hub = 0.5 * (L2 - SR/N)
            nc.vector.tensor_tensor(t0, l2, srn, ALU.subtract)
            nc.scalar.activation(hub, t0, ACTF.Copy, scale=0.5)
            nc.vector.tensor_tensor(l1sq, l1, l1, ALU.mult)
            nc.vector.tensor_scalar(c1, l2, 1.0, None, ALU.is_le)
            nc.vector.tensor_tensor(c2, l2, l1sq, ALU.is_lt)
            nc.vector.tensor_tensor(cond, c1, c2, ALU.max)
            # per = hub + cond * (l2 - hub)
            nc.vector.tensor_tensor(dif, l2, hub, ALU.subtract)
            nc.vector.tensor_tensor(mm, cond, dif, ALU.mult)
            nc.vector.tensor_tensor(per, hub, mm, ALU.add)
            nc.scalar.activation(res[:], per, ACTF.Copy, scale=1.0 / N_CORES)

            cc_in = dram_pool.tile([1, 1], F32)
            cc_out = dram_pool.tile([1, 1], F32)
            nc.gpsimd.dma_start(out=cc_in[:], in_=res[:])
            nc.gpsimd.collective_compute(
                "AllReduce",
                ALU.add,
                replica_groups=[list(range(N_CORES))],
                ins=[cc_in.opt()],
                outs=[cc_out.opt()],
            )
            nc.gpsimd.dma_start(out=out_ext[:, :], in_=cc_out[:])

    nc.compile()
    return nc


_NC_CACHE = {}


def _get_nc():
    if "nc" not in _NC_CACHE:
        _NC_CACHE["nc"] = build()
    return _NC_CACHE["nc"]


def kernel(y_pred_logits: np.ndarray, y_true: np.ndarray, _trace=False) -> np.ndarray:
    nc = _get_nc()
    a = np.ascontiguousarray(y_pred_logits, dtype=np.float32).reshape(N_CORES, P, COLS)
    b = np.ascontiguousarray(y_true, dtype=np.float32).reshape(N_CORES, P, COLS)
    in_maps = [
        {"y_pred_logits": a[i], "y_true": b[i]} for i in range(N_CORES)
    ]
    r = run_bass_kernel_spmd(nc, in_maps, core_ids=list(range(N_CORES)), trace=_trace)
    out = np.asarray(r.results[0]["out"], dtype=np.float32).reshape(())
    if _trace:
        return out, r
    return out


# revision 11
# speedup vs baseline: 4.5352x; 1.2332x over previous
"""Adaptive Huber/MSE/L1 loss on 8 TRN2 NeuronCores (Bass/Tile).

Reference math (per sample, N = 4,096,000 elements):
    e   = pred - true
    L2  = mean(e^2);  L1 = mean(|e|)
    huber_elem = where(|e| <= 5, 0.5 e^2, 5(|e| - 2.5))
               = 0.5 e^2 - 0.5 relu(|e| - 5)^2
    huber = (S2 - SR) * 0.5 / N        (S2 = sum e^2, SR = sum relu(|e|-5)^2)
    use_l2 = (L2 <= 1) | (L2 < L1^2)
    loss = mean_over_batch(where(use_l2, L2, huber))

Sharding: data-parallel, sample i -> core i. Each core reduces its
32.8 MB shard to three sums, applies the branch locally, scales by
1/8, then a 4-byte AllReduce(add) yields the batch mean on every core.

Engine split per [128, F] tile, tuned so both compute engines sit just
under the ~358 GB/s-per-core DMA floor (~94 us for 32.8 MB):
    DVE : e = a - b;  m = max(|e|,5)-5 (2x-mode tensor_scalar);
          bn_stats chunks on EVEN tiles (-> mean/var => partial S2)
    ACT : |e| with row-accum (S1);  Square(m)+row-accum (SR);
          Square(e)+row-accum on ODD tiles (other half of S2)
    PE  : ones^T @ [128,4] partition reduction into PSUM

Hardware pitfalls baked in: DVE tensor_tensor_reduce hangs the device
(avoided); GpSimd elementwise runs ~30 us/tile AND port-starves DVE
(avoided); profiling must capture all 8 devices (see test harness).
"""

import numpy as np

import concourse.bass as bass
import concourse.bacc as bacc
import concourse.mybir as mybir
from concourse.tile import TileContext
from concourse.bass_utils import run_bass_kernel_spmd

P = 128
COLS = 32000  # 160*160*160 / 128
DELTA = 5.0
N_CORES = 8

F32 = mybir.dt.float32
ALU = mybir.AluOpType
ACTF = mybir.ActivationFunctionType


def build(cols=COLS, tile_f=2000):
    assert cols % tile_f == 0
    n_tiles = cols // tile_f
    assert n_tiles % 2 == 0
    chunk = 500 if tile_f % 500 == 0 else tile_f
    assert tile_f % chunk == 0 and chunk <= 512
    n_chunks = tile_f // chunk
    n_elem = float(P * cols)
    bn_elems = float((n_tiles // 2) * tile_f)  # per-partition elems seen by bn_stats

    nc = bacc.Bacc(
        "TRN2",
        target_bir_lowering=False,
        debug=False,
        enable_asserts=False,
        num_devices=N_CORES,
    )
    a_ext = nc.dram_tensor("y_pred_logits", [P, cols], F32, kind="ExternalInput")
    b_ext = nc.dram_tensor("y_true", [P, cols], F32, kind="ExternalInput")
    out_ext = nc.dram_tensor("out", [1, 1], F32, kind="ExternalOutput")

    with TileContext(nc) as tc:
        with (
            tc.tile_pool(name="io", bufs=4) as io_pool,
            tc.tile_pool(name="work", bufs=2) as work_pool,
            tc.tile_pool(name="acc", bufs=1) as acc_pool,
            tc.tile_pool(name="psum", bufs=1, space="PSUM") as psum_pool,
        ):
            stats = acc_pool.tile([P, (n_tiles // 2) * n_chunks, 6], F32)
            sums_sq = acc_pool.tile([P, n_tiles // 2], F32)  # odd tiles via ACT
            sums_ae = acc_pool.tile([P, n_tiles], F32)
            sums_d2 = acc_pool.tile([P, n_tiles], F32)
            ones = acc_pool.tile([P, 1], F32)
            nc.vector.memset(ones[:], 1.0)
            neg_delta = acc_pool.tile([P, 1], F32)
            nc.vector.memset(neg_delta[:], -DELTA)

            for t in range(n_tiles):
                a = io_pool.tile([P, tile_f], F32, tag="a")
                b = io_pool.tile([P, tile_f], F32, tag="b")
                sl = slice(t * tile_f, (t + 1) * tile_f)
                nc.sync.dma_start(out=a[:], in_=a_ext[:, sl])
                nc.sync.dma_start(out=b[:], in_=b_ext[:, sl])

                e = work_pool.tile([P, tile_f], F32, tag="e")
                ae = work_pool.tile([P, tile_f], F32, tag="ae")
                m = work_pool.tile([P, tile_f], F32, tag="m")
                s_d2 = work_pool.tile([P, tile_f], F32, tag="s_d2")

                nc.vector.tensor_tensor(e[:], a[:], b[:], ALU.subtract)
                half = t // 2
                if t % 2 == 0:
                    for c in range(n_chunks):
                        nc.vector.bn_stats(
                            out=stats[:, half * n_chunks + c, :],
                            in_=e[:, c * chunk : (c + 1) * chunk],
                        )
                else:
                    s_sq = work_pool.tile([P, tile_f], F32, tag="s_sq")
                    nc.scalar.activation(
                        s_sq[:], e[:], ACTF.Square,
                        accum_out=sums_sq[:, half : half + 1],
                    )
                nc.scalar.activation(
                    ae[:], e[:], ACTF.Abs, accum_out=sums_ae[:, t : t + 1]
                )
                # m = max(|e|,5) - 5 == relu(|e|-5); 2x-mode tensor_scalar
                nc.vector.tensor_scalar(
                    m[:], ae[:], DELTA, -DELTA, ALU.max, ALU.add
                )
                nc.scalar.activation(
                    s_d2[:], m[:], ACTF.Square,
                    accum_out=sums_d2[:, t : t + 1],
                )

            # per-partition S2 = bn_elems*(var + mean^2)  +  sum(odd-tile squares)
            mv = acc_pool.tile([P, 2], F32)
            nc.vector.bn_aggr(out=mv[:], in_=stats[:])
            red = acc_pool.tile([P, 4], F32)
            tmp = acc_pool.tile([P, 2], F32)
            nc.vector.tensor_tensor(tmp[:, 0:1], mv[:, 0:1], mv[:, 0:1], ALU.mult)
            nc.vector.tensor_tensor(tmp[:, 1:2], tmp[:, 0:1], mv[:, 1:2], ALU.add)
            nc.vector.reduce_sum(red[:, 3:4], sums_sq[:], axis=mybir.AxisListType.X)
            # red0 = bn_elems * tmp1 + red3
            nc.vector.tensor_scalar(tmp[:, 1:2], tmp[:, 1:2], bn_elems, None, ALU.mult)
            nc.vector.tensor_tensor(red[:, 0:1], tmp[:, 1:2], red[:, 3:4], ALU.add)
            nc.vector.reduce_sum(red[:, 1:2], sums_ae[:], axis=mybir.AxisListType.X)
            nc.vector.reduce_sum(red[:, 2:3], sums_d2[:], axis=mybir.AxisListType.X)

            # partition reduce: ps[0,:] = ones^T @ red -> [S2, S1, SR]
            ps = psum_pool.tile([1, 4], F32)
            nc.tensor.matmul(ps[0:1, 0:3], ones[:, 0:1], red[:, 0:3],
                             start=True, stop=True)
            psc = acc_pool.tile([1, 3], F32)
            nc.scalar.activation(psc[:], ps[0:1, 0:3], ACTF.Copy)

            sc = acc_pool.tile([1, 12], F32)
            l2 = sc[:, 0:1]
            l1 = sc[:, 1:2]
            srn = sc[:, 2:3]
            t0 = sc[:, 3:4]
            hub = sc[:, 4:5]
            l1sq = sc[:, 5:6]
            c1 = sc[:, 6:7]
            c2 = sc[:, 7:8]
            cond = sc[:, 8:9]
            dif = sc[:, 9:10]
            mm = sc[:, 10:11]
            per = sc[:, 11:12]

            nc.scalar.activation(l2, psc[:, 0:1], ACTF.Copy, scale=1.0 / n_elem)
            nc.scalar.activation(l1, psc[:, 1:2], ACTF.Copy, scale=1.0 / n_elem)
            nc.scalar.activation(srn, psc[:, 2:3], ACTF.Copy, scale=1.0 / n_elem)
            # hub = 0.5 * (L2 - SR/N)
            nc.vector.tensor_tensor(t0, l2, srn, ALU.subtract)
            nc.scalar.activation(hub, t0, ACTF.Copy, scale=0.5)
            nc.vector.tensor_tensor(l1sq, l1, l1, ALU.mult)
            nc.vector.tensor_scalar(c1, l2, 1.0, None, ALU.is_le)
            nc.vector.tensor_tensor(c2, l2, l1sq, ALU.is_lt)
            nc.vector.tensor_tensor(cond, c1, c2, ALU.max)
            # per = hub + cond * (l2 - hub)
            nc.vector.tensor_tensor(dif, l2, hub, ALU.subtract)
            nc.vector.tensor_tensor(mm, cond, dif, ALU.mult)
            nc.vector.tensor_tensor(per, hub, mm, ALU.add)
            # Each core emits its own per-sample loss; the host averages the
            # 8 scalars during unshard (a 4-byte on-device AllReduce costs
            # ~42 us: ~13 us entry sync + ~29 us mesh latency floor).
            nc.sync.dma_start(out=out_ext[:, :], in_=per)

    nc.compile()
    return nc


_NC_CACHE = {}


def _get_nc():
    if "nc" not in _NC_CACHE:
        _NC_CACHE["nc"] = build()
    return _NC_CACHE["nc"]


def kernel(y_pred_logits: np.ndarray, y_true: np.ndarray, _trace=False) -> np.ndarray:
    nc = _get_nc()
    a = np.ascontiguousarray(y_pred_logits, dtype=np.float32).reshape(N_CORES, P, COLS)
    b = np.ascontiguousarray(y_true, dtype=np.float32).reshape(N_CORES, P, COLS)
    in_maps = [
        {"y_pred_logits": a[i], "y_true": b[i]} for i in range(N_CORES)
    ]
    r = run_bass_kernel_spmd(nc, in_maps, core_ids=list(range(N_CORES)), trace=_trace)
    per_sample = np.array(
        [np.asarray(r.results[i]["out"]).reshape(()) for i in range(N_CORES)],
        dtype=np.float32,
    )
    out = np.float32(per_sample.mean(dtype=np.float32)).reshape(())
    if _trace:
        return out, r
    return out


# revision 15
# speedup vs baseline: 4.9455x; 1.0905x over previous
"""Adaptive Huber/MSE/L1 loss on 8 TRN2 NeuronCores (Bass/Tile).

Reference math (per sample, N = 4,096,000 elements):
    e   = pred - true
    L2  = mean(e^2);  L1 = mean(|e|)
    huber_elem = where(|e| <= 5, 0.5 e^2, 5(|e| - 2.5))
               = 0.5 e^2 - 0.5 relu(|e| - 5)^2
    huber = (S2 - SR) * 0.5 / N        (S2 = sum e^2, SR = sum relu(|e|-5)^2)
    use_l2 = (L2 <= 1) | (L2 < L1^2)
    loss = mean_over_batch(where(use_l2, L2, huber))

Sharding: data-parallel, sample i -> core i. Each core reduces its
32.8 MB shard to three sums, applies the branch locally, scales by
1/8, then a 4-byte AllReduce(add) yields the batch mean on every core.

Engine split per [128, F] tile, tuned so both compute engines sit just
under the ~358 GB/s-per-core DMA floor (~94 us for 32.8 MB):
    DVE : e = a - b;  m = max(|e|,5)-5 (2x-mode tensor_scalar);
          bn_stats chunks on EVEN tiles (-> mean/var => partial S2)
    ACT : |e| with row-accum (S1);  Square(m)+row-accum (SR);
          Square(e)+row-accum on ODD tiles (other half of S2)
    PE  : ones^T @ [128,4] partition reduction into PSUM

Hardware pitfalls baked in: DVE tensor_tensor_reduce hangs the device
(avoided); GpSimd elementwise runs ~30 us/tile AND port-starves DVE
(avoided); profiling must capture all 8 devices (see test harness).
"""

import numpy as np

import concourse.bass as bass
import concourse.bacc as bacc
import concourse.mybir as mybir
from concourse.tile import TileContext
from concourse.bass_utils import run_bass_kernel_spmd

P = 128
COLS = 32000  # 160*160*160 / 128
DELTA = 5.0
N_CORES = 8

F32 = mybir.dt.float32
ALU = mybir.AluOpType
ACTF = mybir.ActivationFunctionType


def build(cols=COLS, tile_f=2000, tail_tiles=4, tail_f=500):
    """Tiles of `tile_f` with the trailing `tail_tiles*tail_f` columns split
    into small tiles, so the last tile's dependency chain (sub -> abs -> max
    -> square) is short and the kernel tail hugs the final DMA."""
    main_cols = cols - tail_tiles * tail_f
    assert main_cols % tile_f == 0 and main_cols > 0
    chunk = 500 if tile_f % 500 == 0 else tile_f
    assert tile_f % chunk == 0 and chunk <= 512 and tail_f <= 512
    tiles = [tile_f] * (main_cols // tile_f) + [tail_f] * tail_tiles
    # even-indexed main tiles go through bn_stats (DVE), the rest through
    # ACT Square; tail tiles all go through ACT (shortest chain).
    use_bn = [i % 2 == 0 and f == tile_f for i, f in enumerate(tiles)]
    n_elem = float(P * cols)
    bn_elems = float(sum(f for f, b in zip(tiles, use_bn) if b))
    n_bn_chunks = sum(f // chunk for f, b in zip(tiles, use_bn) if b)
    n_act_sq = sum(1 for b in use_bn if not b)

    nc = bacc.Bacc(
        "TRN2",
        target_bir_lowering=False,
        debug=False,
        enable_asserts=False,
        num_devices=N_CORES,
    )
    a_ext = nc.dram_tensor("y_pred_logits", [P, cols], F32, kind="ExternalInput")
    b_ext = nc.dram_tensor("y_true", [P, cols], F32, kind="ExternalInput")
    out_ext = nc.dram_tensor("out", [1, 1], F32, kind="ExternalOutput")

    with TileContext(nc) as tc:
        with (
            tc.tile_pool(name="io", bufs=6) as io_pool,
            tc.tile_pool(name="work", bufs=2) as work_pool,
            tc.tile_pool(name="acc", bufs=1) as acc_pool,
            tc.tile_pool(name="psum", bufs=1, space="PSUM") as psum_pool,
        ):
            n_tiles = len(tiles)
            stats = acc_pool.tile([P, n_bn_chunks, 6], F32)
            sums_sq = acc_pool.tile([P, n_act_sq], F32)
            sums_ae = acc_pool.tile([P, n_tiles], F32)
            sums_d2 = acc_pool.tile([P, n_tiles], F32)
            ones = acc_pool.tile([P, 1], F32)
            nc.vector.memset(ones[:], 1.0)
            neg_delta = acc_pool.tile([P, 1], F32)
            nc.vector.memset(neg_delta[:], -DELTA)

            col = 0
            stat_i = 0
            sq_i = 0
            for t, f in enumerate(tiles):
                a = io_pool.tile([P, f], F32, tag="a")
                b = io_pool.tile([P, f], F32, tag="b")
                sl = slice(col, col + f)
                col += f
                nc.sync.dma_start(out=a[:], in_=a_ext[:, sl])
                nc.sync.dma_start(out=b[:], in_=b_ext[:, sl])

                e = work_pool.tile([P, f], F32, tag="e")
                ae = work_pool.tile([P, f], F32, tag="ae")
                m = work_pool.tile([P, f], F32, tag="m")
                s_d2 = work_pool.tile([P, f], F32, tag="s_d2")

                nc.vector.tensor_tensor(e[:], a[:], b[:], ALU.subtract)
                if use_bn[t]:
                    for c in range(f // chunk):
                        nc.vector.bn_stats(
                            out=stats[:, stat_i, :],
                            in_=e[:, c * chunk : (c + 1) * chunk],
                        )
                        stat_i += 1
                else:
                    s_sq = work_pool.tile([P, f], F32, tag="s_sq")
                    nc.scalar.activation(
                        s_sq[:], e[:], ACTF.Square,
                        accum_out=sums_sq[:, sq_i : sq_i + 1],
                    )
                    sq_i += 1
                nc.scalar.activation(
                    ae[:], e[:], ACTF.Abs, accum_out=sums_ae[:, t : t + 1]
                )
                # m = max(|e|,5) - 5 == relu(|e|-5); 2x-mode tensor_scalar
                nc.vector.tensor_scalar(
                    m[:], ae[:], DELTA, -DELTA, ALU.max, ALU.add
                )
                nc.scalar.activation(
                    s_d2[:], m[:], ACTF.Square,
                    accum_out=sums_d2[:, t : t + 1],
                )
            assert stat_i == n_bn_chunks and sq_i == n_act_sq and col == cols

            # per-partition S2 = bn_elems*(var + mean^2)  +  sum(odd-tile squares)
            mv = acc_pool.tile([P, 2], F32)
            nc.vector.bn_aggr(out=mv[:], in_=stats[:])
            red = acc_pool.tile([P, 4], F32)
            tmp = acc_pool.tile([P, 2], F32)
            nc.vector.tensor_tensor(tmp[:, 0:1], mv[:, 0:1], mv[:, 0:1], ALU.mult)
            nc.vector.tensor_tensor(tmp[:, 1:2], tmp[:, 0:1], mv[:, 1:2], ALU.add)
            nc.vector.reduce_sum(red[:, 3:4], sums_sq[:], axis=mybir.AxisListType.X)
            # red0 = bn_elems * tmp1 + red3
            nc.vector.tensor_scalar(tmp[:, 1:2], tmp[:, 1:2], bn_elems, None, ALU.mult)
            nc.vector.tensor_tensor(red[:, 0:1], tmp[:, 1:2], red[:, 3:4], ALU.add)
            nc.vector.reduce_sum(red[:, 1:2], sums_ae[:], axis=mybir.AxisListType.X)
            nc.vector.reduce_sum(red[:, 2:3], sums_d2[:], axis=mybir.AxisListType.X)

            # partition reduce: ps[0,:] = ones^T @ red -> [S2, S1, SR]
            ps = psum_pool.tile([1, 4], F32)
            nc.tensor.matmul(ps[0:1, 0:3], ones[:, 0:1], red[:, 0:3],
                             start=True, stop=True)
            psc = acc_pool.tile([1, 3], F32)
            nc.scalar.activation(psc[:], ps[0:1, 0:3], ACTF.Copy)

            sc = acc_pool.tile([1, 12], F32)
            l2 = sc[:, 0:1]
            l1 = sc[:, 1:2]
            srn = sc[:, 2:3]
            t0 = sc[:, 3:4]
            hub = sc[:, 4:5]
            l1sq = sc[:, 5:6]
            c1 = sc[:, 6:7]
            c2 = sc[:, 7:8]
            cond = sc[:, 8:9]
            dif = sc[:, 9:10]
            mm = sc[:, 10:11]
            per = sc[:, 11:12]

            nc.scalar.activation(l2, psc[:, 0:1], ACTF.Copy, scale=1.0 / n_elem)
            nc.scalar.activation(l1, psc[:, 1:2], ACTF.Copy, scale=1.0 / n_elem)
            nc.scalar.activation(srn, psc[:, 2:3], ACTF.Copy, scale=1.0 / n_elem)
            # hub = 0.5 * (L2 - SR/N)
            nc.vector.tensor_tensor(t0, l2, srn, ALU.subtract)
            nc.scalar.activation(hub, t0, ACTF.Copy, scale=0.5)
            nc.vector.tensor_tensor(l1sq, l1, l1, ALU.mult)
            nc.vector.tensor_scalar(c1, l2, 1.0, None, ALU.is_le)
            nc.vector.tensor_tensor(c2, l2, l1sq, ALU.is_lt)
            nc.vector.tensor_tensor(cond, c1, c2, ALU.max)
            # per = hub + cond * (l2 - hub)
            nc.vector.tensor_tensor(dif, l2, hub, ALU.subtract)
            nc.vector.tensor_tensor(mm, cond, dif, ALU.mult)
            nc.vector.tensor_tensor(per, hub, mm, ALU.add)
            # Each core emits its own per-sample loss; the host averages the
            # 8 scalars during unshard (a 4-byte on-device AllReduce costs
            # ~42 us: ~13 us entry sync + ~29 us mesh latency floor).
            nc.sync.dma_start(out=out_ext[:, :], in_=per)

    nc.compile()
    return nc


_NC_CACHE = {}


def _get_nc():
    if "nc" not in _NC_CACHE:
        _NC_CACHE["nc"] = build()
    return _NC_CACHE["nc"]


def kernel(y_pred_logits: np.ndarray, y_true: np.ndarray, _trace=False) -> np.ndarray:
    nc = _get_nc()
    a = np.ascontiguousarray(y_pred_logits, dtype=np.float32).reshape(N_CORES, P, COLS)
    b = np.ascontiguousarray(y_true, dtype=np.float32).reshape(N_CORES, P, COLS)
    in_maps = [
        {"y_pred_logits": a[i], "y_true": b[i]} for i in range(N_CORES)
    ]
    r = run_bass_kernel_spmd(nc, in_maps, core_ids=list(range(N_CORES)), trace=_trace)
    per_sample = np.array(
        [np.asarray(r.results[i]["out"]).reshape(()) for i in range(N_CORES)],
        dtype=np.float32,
    )
    out = np.float32(per_sample.mean(dtype=np.float32)).reshape(())
    if _trace:
        return out, r
    return out


# revision 16
# speedup vs baseline: 5.1583x; 1.0430x over previous
"""Adaptive Huber/MSE/L1 loss on 8 TRN2 NeuronCores (Bass/Tile).

Reference math (per sample, N = 4,096,000 elements):
    e   = pred - true
    L2  = mean(e^2);  L1 = mean(|e|)
    huber_elem = where(|e| <= 5, 0.5 e^2, 5(|e| - 2.5))
               = 0.5 e^2 - 0.5 relu(|e| - 5)^2
    huber = (S2 - SR) * 0.5 / N        (S2 = sum e^2, SR = sum relu(|e|-5)^2)
    use_l2 = (L2 <= 1) | (L2 < L1^2)
    loss = mean_over_batch(where(use_l2, L2, huber))

Sharding: data-parallel, sample i -> core i. Each core reduces its
32.8 MB shard to three sums and applies the branch locally; the host
averages the 8 per-sample scalars during unshard (an on-device 4-byte
AllReduce costs ~42 us of pure latency).

Per-tile engine split, sized so every engine sits ~20% under the
~358 GB/s-per-core DMA floor (5.86 us per 2 MB tile pair):
    DVE : e = a - b;  |e| = e & 0x7fffffff (uint32 bitcast, 2x mode);
          m = max(|e|,5) - 5 (fused tensor_scalar, 2x mode)
    ACT : Square(e) + row-accum (S2);  Square(m) + row-accum (SR)
    PE  : ones^T @ |e| chunks accumulated in PSUM (S1), plus the final
          [P,2] -> [1,2] partition reduction
The trailing columns use small tiles so the last dependency chain is
short and the kernel tail hugs the final DMA.

Hardware pitfalls baked in: DVE tensor_tensor_reduce and tensor_scalar
abs_max/accum_out fail on this toolchain (avoided); GpSimd elementwise
runs ~30 us/tile AND port-starves DVE (avoided); profiling must capture
all 8 devices (see test harness).
"""

import numpy as np

import concourse.bass as bass
import concourse.bacc as bacc
import concourse.mybir as mybir
from concourse.tile import TileContext
from concourse.bass_utils import run_bass_kernel_spmd

P = 128
COLS = 32000  # 160*160*160 / 128
DELTA = 5.0
N_CORES = 8
CHUNK = 500  # PE reduction column-chunk (PSUM bank limit 512)

F32 = mybir.dt.float32
U32 = mybir.dt.uint32
ALU = mybir.AluOpType
ACTF = mybir.ActivationFunctionType


def build(cols=COLS, tile_f=2000, tail=(1000, 500, 500)):
    main_cols = cols - sum(tail)
    assert main_cols % tile_f == 0 and main_cols > 0
    tiles = [tile_f] * (main_cols // tile_f) + list(tail)
    assert all(f % CHUNK == 0 or f < CHUNK for f in tiles)
    n_elem = float(P * cols)
    n_tiles = len(tiles)
    total_mm = sum(max(1, f // CHUNK) for f in tiles)

    nc = bacc.Bacc(
        "TRN2",
        target_bir_lowering=False,
        debug=False,
        enable_asserts=False,
        num_devices=N_CORES,
    )
    a_ext = nc.dram_tensor("y_pred_logits", [P, cols], F32, kind="ExternalInput")
    b_ext = nc.dram_tensor("y_true", [P, cols], F32, kind="ExternalInput")
    out_ext = nc.dram_tensor("out", [1, 1], F32, kind="ExternalOutput")

    with TileContext(nc) as tc:
        with (
            tc.tile_pool(name="io", bufs=6) as io_pool,
            tc.tile_pool(name="work", bufs=2) as work_pool,
            tc.tile_pool(name="acc", bufs=1) as acc_pool,
            tc.tile_pool(name="psum", bufs=1, space="PSUM") as psum_pool,
        ):
            sums_sq = acc_pool.tile([P, n_tiles], F32)
            sums_d2 = acc_pool.tile([P, n_tiles], F32)
            ones = acc_pool.tile([P, 1], F32)
            nc.vector.memset(ones[:], 1.0)
            psum_ae = psum_pool.tile([1, CHUNK], F32)

            col = 0
            mm_i = 0
            for t, f in enumerate(tiles):
                a = io_pool.tile([P, f], F32, tag="a")
                b = io_pool.tile([P, f], F32, tag="b")
                sl = slice(col, col + f)
                col += f
                nc.sync.dma_start(out=a[:], in_=a_ext[:, sl])
                nc.sync.dma_start(out=b[:], in_=b_ext[:, sl])

                e = work_pool.tile([P, f], F32, tag="e")
                ae = work_pool.tile([P, f], F32, tag="ae")
                m = work_pool.tile([P, f], F32, tag="m")
                s_sq = work_pool.tile([P, f], F32, tag="s_sq")
                s_d2 = work_pool.tile([P, f], F32, tag="s_d2")

                nc.vector.tensor_tensor(e[:], a[:], b[:], ALU.subtract)
                nc.vector.tensor_scalar(
                    ae.bitcast(U32)[:], e.bitcast(U32)[:],
                    0x7FFFFFFF, None, ALU.bitwise_and,
                )
                # m = max(|e|,5) - 5 == relu(|e|-5); 2x-mode tensor_scalar
                nc.vector.tensor_scalar(
                    m[:], ae[:], DELTA, -DELTA, ALU.max, ALU.add
                )
                nc.scalar.activation(
                    s_sq[:], e[:], ACTF.Square,
                    accum_out=sums_sq[:, t : t + 1],
                )
                nc.scalar.activation(
                    s_d2[:], m[:], ACTF.Square,
                    accum_out=sums_d2[:, t : t + 1],
                )
                for c in range(max(1, f // CHUNK)):
                    w = min(CHUNK, f - c * CHUNK)
                    nc.tensor.matmul(
                        psum_ae[0:1, 0:w], ones[:, 0:1],
                        ae[:, c * CHUNK : c * CHUNK + w],
                        start=(mm_i == 0), stop=(mm_i == total_mm - 1),
                    )
                    mm_i += 1
            assert col == cols and mm_i == total_mm

            red = acc_pool.tile([P, 2], F32)
            nc.vector.reduce_sum(red[:, 0:1], sums_sq[:], axis=mybir.AxisListType.X)
            nc.vector.reduce_sum(red[:, 1:2], sums_d2[:], axis=mybir.AxisListType.X)

            # partition reduce: ps[0,:] = ones^T @ red -> [S2, SR]
            ps = psum_pool.tile([1, 2], F32)
            nc.tensor.matmul(ps[0:1, 0:2], ones[:, 0:1], red[:, 0:2],
                             start=True, stop=True)

            sc = acc_pool.tile([1, 12], F32)
            l2 = sc[:, 0:1]
            srn = sc[:, 1:2]
            l1 = sc[:, 2:3]
            s1r = sc[:, 3:4]
            hub = sc[:, 4:5]
            l1sq = sc[:, 5:6]
            c1 = sc[:, 6:7]
            c2 = sc[:, 7:8]
            cond = sc[:, 8:9]
            dif = sc[:, 9:10]
            mm = sc[:, 10:11]
            per = sc[:, 11:12]

            # [L2, SR/N] in one scaled copy; S1 via the PE accumulator
            nc.scalar.activation(sc[:, 0:2], ps[0:1, 0:2], ACTF.Copy,
                                 scale=1.0 / n_elem)
            nc.vector.reduce_sum(s1r, psum_ae[0:1, :], axis=mybir.AxisListType.X)
            nc.scalar.activation(l1, s1r, ACTF.Copy, scale=1.0 / n_elem)
            # hub = 0.5 * (L2 - SR/N)
            nc.vector.tensor_tensor(dif, l2, srn, ALU.subtract)
            nc.scalar.activation(hub, dif, ACTF.Copy, scale=0.5)
            nc.vector.tensor_tensor(l1sq, l1, l1, ALU.mult)
            nc.vector.tensor_scalar(c1, l2, 1.0, None, ALU.is_le)
            nc.vector.tensor_tensor(c2, l2, l1sq, ALU.is_lt)
            nc.vector.tensor_tensor(cond, c1, c2, ALU.max)
            # per = hub + cond * (l2 - hub)
            nc.vector.tensor_tensor(dif, l2, hub, ALU.subtract)
            nc.vector.tensor_tensor(mm, cond, dif, ALU.mult)
            nc.vector.tensor_tensor(per, hub, mm, ALU.add)
            # Each core emits its own per-sample loss; the host averages the
            # 8 scalars during unshard.
            nc.sync.dma_start(out=out_ext[:, :], in_=per)

    nc.compile()
    return nc


_NC_CACHE = {}


def _get_nc():
    if "nc" not in _NC_CACHE:
        _NC_CACHE["nc"] = build()
    return _NC_CACHE["nc"]


def kernel(y_pred_logits: np.ndarray, y_true: np.ndarray, _trace=False) -> np.ndarray:
    nc = _get_nc()
    a = np.ascontiguousarray(y_pred_logits, dtype=np.float32).reshape(N_CORES, P, COLS)
    b = np.ascontiguousarray(y_true, dtype=np.float32).reshape(N_CORES, P, COLS)
    in_maps = [
        {"y_pred_logits": a[i], "y_true": b[i]} for i in range(N_CORES)
    ]
    r = run_bass_kernel_spmd(nc, in_maps, core_ids=list(range(N_CORES)), trace=_trace)
    per_sample = np.array(
        [np.asarray(r.results[i]["out"]).reshape(()) for i in range(N_CORES)],
        dtype=np.float32,
    )
    out = np.float32(per_sample.mean(dtype=np.float32)).reshape(())
    if _trace:
        return out, r
    return out


# revision 18
# speedup vs baseline: 5.1684x; 1.0020x over previous
"""Adaptive Huber/MSE/L1 loss on 8 TRN2 NeuronCores (Bass/Tile).

Reference math (per sample, N = 4,096,000 elements):
    e   = pred - true
    L2  = mean(e^2);  L1 = mean(|e|)
    huber_elem = where(|e| <= 5, 0.5 e^2, 5(|e| - 2.5))
               = 0.5 e^2 - 0.5 relu(|e| - 5)^2
    huber = (S2 - SR) * 0.5 / N        (S2 = sum e^2, SR = sum relu(|e|-5)^2)
    use_l2 = (L2 <= 1) | (L2 < L1^2)
    loss = mean_over_batch(where(use_l2, L2, huber))

Sharding: data-parallel, sample i -> core i. Each core reduces its
32.8 MB shard to three sums and applies the branch locally; the host
averages the 8 per-sample scalars during unshard (an on-device 4-byte
AllReduce costs ~42 us of pure latency).

Per-tile engine split, sized so every engine sits ~20% under the
~358 GB/s-per-core DMA floor (5.86 us per 2 MB tile pair):
    DVE : e = a - b;  |e| = e & 0x7fffffff (uint32 bitcast, 2x mode);
          m = max(|e|,5) - 5 (fused tensor_scalar, 2x mode)
    ACT : Square(e) + row-accum (S2);  Square(m) + row-accum (SR)
    PE  : ones^T @ |e| chunks accumulated in PSUM (S1), plus the final
          [P,2] -> [1,2] partition reduction
The trailing columns use small tiles so the last dependency chain is
short and the kernel tail hugs the final DMA.

Hardware pitfalls baked in: DVE tensor_tensor_reduce and tensor_scalar
abs_max/accum_out fail on this toolchain (avoided); GpSimd elementwise
runs ~30 us/tile AND port-starves DVE (avoided); profiling must capture
all 8 devices (see test harness).
"""

import numpy as np

import concourse.bass as bass
import concourse.bacc as bacc
import concourse.mybir as mybir
from concourse.tile import TileContext
from concourse.bass_utils import run_bass_kernel_spmd

P = 128
COLS = 32000  # 160*160*160 / 128
DELTA = 5.0
N_CORES = 8
CHUNK = 500  # PE reduction column-chunk (PSUM bank limit 512)

F32 = mybir.dt.float32
U32 = mybir.dt.uint32
ALU = mybir.AluOpType
ACTF = mybir.ActivationFunctionType


def build(cols=COLS, tile_f=2000, tail=(1000, 500, 500)):
    main_cols = cols - sum(tail)
    assert main_cols % tile_f == 0 and main_cols > 0
    tiles = [tile_f] * (main_cols // tile_f) + list(tail)
    assert all(f % CHUNK == 0 or f < CHUNK for f in tiles)
    n_elem = float(P * cols)
    n_tiles = len(tiles)
    total_mm = sum(max(1, f // CHUNK) for f in tiles)
    w_max = min(CHUNK, max(tiles))
    # first matmul carries start=True and must reset the widest PSUM region
    assert min(CHUNK, tiles[0]) == w_max

    nc = bacc.Bacc(
        "TRN2",
        target_bir_lowering=False,
        debug=False,
        enable_asserts=False,
        num_devices=N_CORES,
    )
    a_ext = nc.dram_tensor("y_pred_logits", [P, cols], F32, kind="ExternalInput")
    b_ext = nc.dram_tensor("y_true", [P, cols], F32, kind="ExternalInput")
    out_ext = nc.dram_tensor("out", [1, 1], F32, kind="ExternalOutput")

    with TileContext(nc) as tc:
        with (
            tc.tile_pool(name="io", bufs=6) as io_pool,
            tc.tile_pool(name="work", bufs=2) as work_pool,
            tc.tile_pool(name="acc", bufs=1) as acc_pool,
            tc.tile_pool(name="psum", bufs=1, space="PSUM") as psum_pool,
        ):
            sums_sq = acc_pool.tile([P, n_tiles], F32)
            sums_d2 = acc_pool.tile([P, n_tiles], F32)
            ones = acc_pool.tile([P, 1], F32)
            nc.vector.memset(ones[:], 1.0)
            psum_ae = psum_pool.tile([1, w_max], F32)

            col = 0
            mm_i = 0
            for t, f in enumerate(tiles):
                a = io_pool.tile([P, f], F32, tag="a")
                b = io_pool.tile([P, f], F32, tag="b")
                sl = slice(col, col + f)
                col += f
                nc.sync.dma_start(out=a[:], in_=a_ext[:, sl])
                nc.sync.dma_start(out=b[:], in_=b_ext[:, sl])

                e = work_pool.tile([P, f], F32, tag="e")
                ae = work_pool.tile([P, f], F32, tag="ae")
                m = work_pool.tile([P, f], F32, tag="m")
                s_sq = work_pool.tile([P, f], F32, tag="s_sq")
                s_d2 = work_pool.tile([P, f], F32, tag="s_d2")

                nc.vector.tensor_tensor(e[:], a[:], b[:], ALU.subtract)
                nc.vector.tensor_scalar(
                    ae.bitcast(U32)[:], e.bitcast(U32)[:],
                    0x7FFFFFFF, None, ALU.bitwise_and,
                )
                # m = max(|e|,5) - 5 == relu(|e|-5); 2x-mode tensor_scalar
                nc.vector.tensor_scalar(
                    m[:], ae[:], DELTA, -DELTA, ALU.max, ALU.add
                )
                nc.scalar.activation(
                    s_sq[:], e[:], ACTF.Square,
                    accum_out=sums_sq[:, t : t + 1],
                )
                nc.scalar.activation(
                    s_d2[:], m[:], ACTF.Square,
                    accum_out=sums_d2[:, t : t + 1],
                )
                for c in range(max(1, f // CHUNK)):
                    w = min(CHUNK, f - c * CHUNK)
                    nc.tensor.matmul(
                        psum_ae[0:1, 0:w], ones[:, 0:1],
                        ae[:, c * CHUNK : c * CHUNK + w],
                        start=(mm_i == 0), stop=(mm_i == total_mm - 1),
                    )
                    mm_i += 1
            assert col == cols and mm_i == total_mm

            red = acc_pool.tile([P, 2], F32)
            nc.vector.reduce_sum(red[:, 0:1], sums_sq[:], axis=mybir.AxisListType.X)
            nc.vector.reduce_sum(red[:, 1:2], sums_d2[:], axis=mybir.AxisListType.X)

            # partition reduce: ps[0,:] = ones^T @ red -> [S2, SR]
            ps = psum_pool.tile([1, 2], F32)
            nc.tensor.matmul(ps[0:1, 0:2], ones[:, 0:1], red[:, 0:2],
                             start=True, stop=True)

            sc = acc_pool.tile([1, 12], F32)
            l2 = sc[:, 0:1]
            srn = sc[:, 1:2]
            l1 = sc[:, 2:3]
            s1r = sc[:, 3:4]
            hub = sc[:, 4:5]
            l1sq = sc[:, 5:6]
            c1 = sc[:, 6:7]
            c2 = sc[:, 7:8]
            cond = sc[:, 8:9]
            dif = sc[:, 9:10]
            mm = sc[:, 10:11]
            per = sc[:, 11:12]

            # [L2, SR/N] in one scaled copy; S1 via the PE accumulator
            nc.scalar.activation(sc[:, 0:2], ps[0:1, 0:2], ACTF.Copy,
                                 scale=1.0 / n_elem)
            nc.vector.reduce_sum(s1r, psum_ae[0:1, :], axis=mybir.AxisListType.X)
            nc.scalar.activation(l1, s1r, ACTF.Copy, scale=1.0 / n_elem)
            # hub = 0.5 * (L2 - SR/N)
            nc.vector.tensor_tensor(dif, l2, srn, ALU.subtract)
            nc.scalar.activation(hub, dif, ACTF.Copy, scale=0.5)
            nc.vector.tensor_tensor(l1sq, l1, l1, ALU.mult)
            nc.vector.tensor_scalar(c1, l2, 1.0, None, ALU.is_le)
            nc.vector.tensor_tensor(c2, l2, l1sq, ALU.is_lt)
            nc.vector.tensor_tensor(cond, c1, c2, ALU.max)
            # per = hub + cond * (l2 - hub)
            nc.vector.tensor_tensor(dif, l2, hub, ALU.subtract)
            nc.vector.tensor_tensor(mm, cond, dif, ALU.mult)
            nc.vector.tensor_tensor(per, hub, mm, ALU.add)
            # Each core emits its own per-sample loss; the host averages the
            # 8 scalars during unshard.
            nc.sync.dma_start(out=out_ext[:, :], in_=per)

    nc.compile()
    return nc


_NC_CACHE = {}


def _get_nc():
    if "nc" not in _NC_CACHE:
        _NC_CACHE["nc"] = build()
    return _NC_CACHE["nc"]


def kernel(y_pred_logits: np.ndarray, y_true: np.ndarray, _trace=False) -> np.ndarray:
    nc = _get_nc()
    a = np.ascontiguousarray(y_pred_logits, dtype=np.float32).reshape(N_CORES, P, COLS)
    b = np.ascontiguousarray(y_true, dtype=np.float32).reshape(N_CORES, P, COLS)
    in_maps = [
        {"y_pred_logits": a[i], "y_true": b[i]} for i in range(N_CORES)
    ]
    r = run_bass_kernel_spmd(nc, in_maps, core_ids=list(range(N_CORES)), trace=_trace)
    per_sample = np.array(
        [np.asarray(r.results[i]["out"]).reshape(()) for i in range(N_CORES)],
        dtype=np.float32,
    )
    out = np.float32(per_sample.mean(dtype=np.float32)).reshape(())
    if _trace:
        return out, r
    return out
